# revision 2
# baseline (speedup 1.0000x reference)
"""Causal self-attention (B=4, T=2048, C=1024, H=16) on 8 trn2 NeuronCores — v2.

Sharding: core = (batch b, head-group hg), b = core//2, hg = core%2; Megatron
column-parallel qkv / row-parallel proj; host sums the two partial outputs.

v2 layout/schedule choices:
  - all matmul operands bf16 (host pre-converts); fp32 PSUM accumulation.
    Output DMA'd as bf16, host converts+sums.
  - causal handled by ragged diagonal chunks: S^T computed per [128 kv x n q]
    with n = 512-128r on the diagonal — no dead compute, no dead exp, no
    memsets. A 0/1 triangle multiply fixes the single straddling 128-strip.
  - exp is the only ACT work (one instruction per kv chunk, both heads via a
    strided [128,2,n] AP); PSUM->SBUF copies and masks on DVE; softmax
    reciprocal broadcast via Pool-engine partition_broadcast.
  - software pipelining: V-projection chunks and the NEXT pair's Q/K
    projection groups are emitted as PE filler inside each pair's attention
    chunk loop, so PE stays dense while ACT exps (keeps HAM warm).
  - PSUM budget (8 banks): 2 acc (proj) + 2x2 S tiles + 2 y accumulators.
"""

import functools

import numpy as np

B, T, C, H = 4, 2048, 1024, 16
HD = C // H  # 64
N_CORES = 8
HG = 2  # head groups
NH = H // HG  # heads per core = 8
NP = NH // 2  # head pairs per core = 4
TT = T // 128  # 16 kv/t chunks
TB = T // 512  # 4 q blocks
CK = C // 128  # 8 c-chunks


def _build(rep=1):
    import concourse.mybir as mybir
    import concourse.tile as tile
    from concourse import bacc

    f32 = mybir.dt.float32
    bf16 = mybir.dt.bfloat16
    EXP = mybir.ActivationFunctionType.Exp

    nc = bacc.Bacc("TRN2", target_bir_lowering=False, debug=False)

    xt_d = nc.dram_tensor("xt", [C, T], bf16, kind="ExternalInput")
    wq_d = nc.dram_tensor("wq", [C, 512], bf16, kind="ExternalInput")
    wk_d = nc.dram_tensor("wk", [C, 512], bf16, kind="ExternalInput")
    wv_d = nc.dram_tensor("wv", [C, 512], bf16, kind="ExternalInput")
    wp_d = nc.dram_tensor("wp", [512, C], bf16, kind="ExternalInput")
    tri_d = nc.dram_tensor("tri", [128, 128], bf16, kind="ExternalInput")
    out_d = nc.dram_tensor("out", [T, C], bf16, kind="ExternalOutput")

    with tile.TileContext(nc) as tc:
        with tc.tile_pool(name="persist", bufs=1) as persist:
            xt_sb = persist.tile([128, CK, T], bf16, tag="xt")
            wq_sb = persist.tile([128, CK, 512], bf16, tag="wq")
            wk_sb = persist.tile([128, CK, 512], bf16, tag="wk")
            wv_sb = persist.tile([128, CK, 512], bf16, tag="wv")
            wp_sb = persist.tile([128, NP, C], bf16, tag="wp")
            tri_sb = persist.tile([128, 128], bf16, tag="tri")
            qt_sb = persist.tile([128, NP, T], bf16, tag="qt")
            kt_sb = persist.tile([128, NP, T], bf16, tag="kt")
            v_sb = persist.tile([128, TT, NH, HD + 1], bf16, tag="v")
            yt_sb = persist.tile([128, NP, T], bf16, tag="yt")

            def body():
                # ones column of V' (only column HD; V overlays 0:HD)
                nc.vector.memset(v_sb[:, :, :, HD : HD + 1], 1.0)
                # DMAs in consumption order; xt split by t-quarter so the
                # first V-projection chunks can start early
                nc.sync.dma_start(tri_sb[:], tri_d[:, :])
                for k in range(CK):
                    ksl = slice(k * 128, (k + 1) * 128)
                    nc.sync.dma_start(wv_sb[:, k, :], wv_d[ksl, :])
                for tq in range(4):
                    tsl = slice(tq * 512, (tq + 1) * 512)
                    for k in range(CK):
                        ksl = slice(k * 128, (k + 1) * 128)
                        nc.sync.dma_start(xt_sb[:, k, tsl], xt_d[ksl, tsl])
                for k in range(CK):
                    ksl = slice(k * 128, (k + 1) * 128)
                    nc.sync.dma_start(wq_sb[:, k, :], wq_d[ksl, :])
                    nc.sync.dma_start(wk_sb[:, k, :], wk_d[ksl, :])
                nc.sync.dma_start(
                    wp_sb[:], wp_d.rearrange("(a p) n -> p a n", p=128)
                )

                with (
                    tc.tile_pool(name="ptp", bufs=6) as ptp,
                    tc.tile_pool(name="recp", bufs=4) as recp,
                    tc.tile_pool(name="outp", bufs=4) as outp,
                    tc.tile_pool(name="psacc", bufs=1, space="PSUM") as psacc,
                    tc.tile_pool(name="pss", bufs=2, space="PSUM") as pss,
                    tc.tile_pool(name="psya", bufs=2, space="PSUM") as psya,
                    tc.tile_pool(name="psyb", bufs=1, space="PSUM") as psyb,
                ):
                    # warm the exp table before any real dependency exists
                    warm = recp.tile([1, 8], f32, tag="warm", name="warm_in")
                    warm2 = recp.tile([1, 8], f32, tag="warm2", name="warm_out")
                    nc.vector.memset(warm[:], 0.0)
                    nc.scalar.activation(warm2[:], warm[:], EXP, scale=1.0)

                    # ---- emitters ----------------------------------------
                    # B and D cycle the "s"-tag [128,1024] slots (pss) so the
                    # single psacc bank is enough for the A-filler groups.
                    def emit_b(tt):  # V projection chunk tt
                        psv = pss.tile([128, 1024], f32, tag="s", name=f"psv{tt}")
                        for k in range(CK):
                            nc.tensor.matmul(
                                psv[:, 0:512],
                                xt_sb[:, k, tt * 128 : (tt + 1) * 128],
                                wv_sb[:, k, :],
                                start=(k == 0), stop=(k == CK - 1),
                            )
                        nc.vector.tensor_copy(
                            v_sb[:, tt, :, 0:HD],
                            psv[:, 0:512].rearrange("p (h e) -> p h e", e=HD),
                        )

                    def emit_a(p, tb, which):  # Q/K projection group
                        w_sb, dst = (wq_sb, qt_sb) if which == 0 else (wk_sb, kt_sb)
                        ps = psacc.tile(
                            [128, 512], f32, tag="acc", name=f"ps{which}_{p}_{tb}"
                        )
                        for k in range(CK):
                            nc.tensor.matmul(
                                ps[:],
                                w_sb[:, k, p * 128 : (p + 1) * 128],
                                xt_sb[:, k, tb * 512 : (tb + 1) * 512],
                                start=(k == 0), stop=(k == CK - 1),
                            )
                        nc.vector.tensor_copy(dst[:, p, tb * 512 : (tb + 1) * 512], ps[:])

                    def chunks_of():
                        return [(qb, c) for qb in range(TB) for c in range(4 * qb + 4)]

                    pt_store = {}
                    y_store = {}

                    def emit_s(p, qb, c):
                        r = c - 4 * qb  # >= 0: diagonal chunk
                        n = 512 if r < 0 else 512 - 128 * r
                        qoff = 512 * qb + (512 - n)
                        ksl = slice(c * 128, (c + 1) * 128)
                        s2 = pss.tile([128, 1024], f32, tag="s", name=f"s2_{p}_{qb}_{c}")
                        # head a -> bank 0 [0:n], head b -> bank 1 [512:512+n]
                        nc.tensor.matmul(
                            s2[:, 0:n],
                            kt_sb[0:64, p, ksl],
                            qt_sb[0:64, p, qoff : qoff + n],
                            start=True, stop=True,
                        )
                        nc.tensor.matmul(
                            s2[:, 512 : 512 + n],
                            kt_sb[64:128, p, ksl],
                            qt_sb[64:128, p, qoff : qoff + n],
                            start=True, stop=True,
                        )
                        pt = ptp.tile([128, 1024], bf16, tag="pt", name=f"pt_{p}_{qb}_{c}")
                        s2v = s2[:].rearrange("p (b x) -> p b x", b=2)[:, :, 0:n]
                        ptv = pt[:].rearrange("p (b x) -> p b x", b=2)[:, :, 0:n]
                        nc.scalar.activation(ptv, s2v, EXP, scale=0.125)
                        if r >= 0:
                            nc.vector.tensor_mul(pt[:, 0:128], pt[:, 0:128], tri_sb[:])
                            nc.vector.tensor_mul(
                                pt[:, 512:640], pt[:, 512:640], tri_sb[:]
                            )
                        pt_store[(p, qb, c)] = (pt, n)

                    def emit_av(p, qb, c):
                        pt, n = pt_store.pop((p, qb, c))
                        qloc = 512 - n
                        if (p, qb) not in y_store:
                            ya = psya.tile([65, 512], f32, tag="ya", name=f"ya_{p}_{qb}")
                            yb = psyb.tile([65, 512], f32, tag="yb", name=f"yb_{p}_{qb}")
                            y_store[(p, qb)] = (ya, yb)
                        ya, yb = y_store[(p, qb)]
                        st, sp = c == 0, c == 4 * qb + 3
                        nc.tensor.matmul(
                            ya[:, qloc:512], v_sb[:, c, 2 * p, :], pt[:, 0:n],
                            start=st, stop=sp,
                        )
                        nc.tensor.matmul(
                            yb[:, qloc:512], v_sb[:, c, 2 * p + 1, :],
                            pt[:, 512 : 512 + n],
                            start=st, stop=sp,
                        )
                        if not sp:
                            return
                        qsl = slice(qb * 512, (qb + 1) * 512)
                        for hi, yy in ((0, ya), (1, yb)):
                            ri = recp.tile(
                                [1, 512], f32, tag="ri", name=f"ri_{p}_{qb}_{hi}"
                            )
                            nc.vector.tensor_copy(ri[:], yy[64:65, :])
                            rec = recp.tile(
                                [1, 512], f32, tag="rec", name=f"rec_{p}_{qb}_{hi}"
                            )
                            nc.vector.reciprocal_approx_fast(rec[:], ri[:])
                            bc = recp.tile(
                                [64, 512], f32, tag="bc", name=f"bc_{p}_{qb}_{hi}"
                            )
                            nc.gpsimd.partition_broadcast(bc[:], rec[:])
                            nc.vector.tensor_mul(
                                yt_sb[hi * 64 : (hi + 1) * 64, p, qsl],
                                yy[0:64, :], bc[:],
                            )
                        del y_store[(p, qb)]

                    def emit_d(tt, nb):
                        if (2 * tt + nb) % 3 == 2:
                            pot = psacc.tile(
                                [128, 512], f32, tag="acc", name=f"po{tt}_{nb}"
                            )
                            po = pot[:]
                        else:
                            pot = pss.tile(
                                [128, 1024], f32, tag="s", name=f"po{tt}_{nb}"
                            )
                            po = pot[:, 0:512]
                        for p in range(NP):
                            nc.tensor.matmul(
                                po,
                                yt_sb[:, p, tt * 128 : (tt + 1) * 128],
                                wp_sb[:, p, nb * 512 : (nb + 1) * 512],
                                start=(p == 0), stop=(p == NP - 1),
                            )
                        ot = outp.tile([128, 512], bf16, tag="ot", name=f"ot{tt}_{nb}")
                        if (2 * tt + nb) % 2:
                            nc.scalar.copy(ot[:], po)
                        else:
                            nc.vector.tensor_copy(ot[:], po)
                        nc.sync.dma_start(
                            out_d[tt * 128 : (tt + 1) * 128, nb * 512 : (nb + 1) * 512],
                            ot[:],
                        )

                    # ---- schedule emission -------------------------------
                    LA = 2

                    def run_pair(p, fillers):
                        cl = chunks_of()
                        drop = {}
                        if fillers:
                            for i, f in enumerate(fillers):
                                drop.setdefault(int(i * len(cl) / len(fillers)), []).append(f)
                        for j in range(min(LA, len(cl))):
                            emit_s(p, *cl[j])
                        for i in range(len(cl)):
                            for f in drop.get(i, ()):
                                f()
                            if i + LA < len(cl):
                                emit_s(p, *cl[i + LA])
                            emit_av(p, *cl[i])

                    # prolog: alternate V chunks with pair-0 Q/K groups so the
                    # two streams ping-pong the psum slots without stalling
                    for j in range(8):
                        emit_b(j)
                        emit_a(0, j // 2, j % 2)
                    run_pair(
                        0,
                        [functools.partial(emit_b, tt) for tt in range(8, TT)]
                        + [
                            functools.partial(emit_a, 1, tb, w)
                            for tb in range(TB) for w in (0, 1)
                        ],
                    )
                    run_pair(
                        1,
                        [
                            functools.partial(emit_a, 2, tb, w)
                            for tb in range(TB) for w in (0, 1)
                        ],
                    )
                    run_pair(
                        2,
                        [
                            functools.partial(emit_a, 3, tb, w)
                            for tb in range(TB) for w in (0, 1)
                        ],
                    )
                    run_pair(3, [])
                    for tt in range(TT):
                        for nb in range(2):
                            emit_d(tt, nb)

            if rep == 1:
                body()
            else:
                with tc.For_i(0, rep, 1):
                    body()

    nc.compile()
    return nc


@functools.lru_cache(maxsize=None)
def _get_nc(rep=1):
    return _build(rep)


def make_in_maps(x, w_qkv, w_proj):
    import ml_dtypes

    bf = ml_dtypes.bfloat16
    tri = (np.arange(128)[None, :] >= np.arange(128)[:, None]).astype(bf)
    in_maps = []
    for core in range(N_CORES):
        b, hg = divmod(core, HG)
        sl = slice(hg * 512, (hg + 1) * 512)
        in_maps.append({
            "xt": np.ascontiguousarray(x[b].T).astype(bf),
            "wq": np.ascontiguousarray(w_qkv[sl].T).astype(bf),
            "wk": np.ascontiguousarray(w_qkv[C : 2 * C][sl].T).astype(bf),
            "wv": np.ascontiguousarray(w_qkv[2 * C : 3 * C][sl].T).astype(bf),
            "wp": np.ascontiguousarray(w_proj[:, sl].T).astype(bf),
            "tri": tri,
        })
    return in_maps


def combine(results):
    out = np.empty((B, T, C), dtype=np.float32)
    for b in range(B):
        out[b] = results[2 * b]["out"].astype(np.float32) + results[
            2 * b + 1
        ]["out"].astype(np.float32)
    return out


# ---------------------------------------------------------------------------
# PJRT runner (device-resident inputs, reusable jitted executable)
# ---------------------------------------------------------------------------

class _Runner:
    def __init__(self, nc, n_cores=N_CORES):
        import jax
        import concourse.mybir as mybir
        from concourse import bass2jax
        from jax.sharding import Mesh, PartitionSpec, NamedSharding
        from jax.experimental.shard_map import shard_map

        self.jax = jax
        bass2jax.install_neuronx_cc_hook()
        partition_name = (
            nc.partition_id_tensor.name if nc.partition_id_tensor else None
        )
        in_names, out_names, out_avals, zero_outs = [], [], [], []
        for alloc in nc.m.functions[0].allocations:
            if not isinstance(alloc, mybir.MemoryLocationSet):
                continue
            name = alloc.memorylocations[0].name
            if alloc.kind == "ExternalInput":
                if name != partition_name:
                    in_names.append(name)
            elif alloc.kind == "ExternalOutput":
                out_names.append(name)
                shape = tuple(alloc.tensor_shape)
                dtype = mybir.dt.np(alloc.dtype)
                out_avals.append(jax.core.ShapedArray(shape, dtype))
                zero_outs.append(np.zeros(shape, dtype))
        self.in_names, self.out_names = in_names, out_names
        self.out_avals, self.zero_outs = out_avals, zero_outs
        self.n_cores = n_cores
        all_names = in_names + out_names
        if partition_name is not None:
            all_names = all_names + [partition_name]

        def _bdy(*args):
            operands = list(args)
            if partition_name is not None:
                operands.append(bass2jax.partition_id_tensor())
            outs = bass2jax._bass_exec_p.bind(
                *operands,
                out_avals=tuple(out_avals),
                in_names=tuple(all_names),
                out_names=tuple(out_names),
                lowering_input_output_aliases=(),
                sim_require_finite=True,
                sim_require_nnan=True,
                nc=nc,
            )
            return tuple(outs)

        devices = jax.devices()[:n_cores]
        mesh = Mesh(np.asarray(devices), ("core",))
        n_args = len(in_names) + len(out_names)
        self.fn = jax.jit(
            shard_map(
                _bdy, mesh=mesh,
                in_specs=(PartitionSpec("core"),) * n_args,
                out_specs=(PartitionSpec("core"),) * len(out_names),
                check_rep=False,
            ),
            keep_unused=True,
        )
        self.sharding = NamedSharding(mesh, PartitionSpec("core"))

    def put_inputs(self, in_maps):
        concat = [
            np.concatenate([np.asarray(m[name]) for m in in_maps], axis=0)
            for name in self.in_names
        ]
        concat += [
            np.zeros((self.n_cores * z.shape[0], *z.shape[1:]), z.dtype)
            for z in self.zero_outs
        ]
        self.args = [self.jax.device_put(a, self.sharding) for a in concat]
        self.jax.block_until_ready(self.args)

    def run(self):
        outs = self.fn(*self.args)
        self.jax.block_until_ready(outs)
        return [
            {
                name: np.asarray(outs[i]).reshape(
                    self.n_cores, *self.out_avals[i].shape)[c]
                for i, name in enumerate(self.out_names)
            }
            for c in range(self.n_cores)
        ]

    def time_ns(self, iters=20, warmup=2):
        import time
        for _ in range(warmup):
            self.jax.block_until_ready(self.fn(*self.args))
        t0 = time.perf_counter()
        outs = None
        for _ in range(iters):
            outs = self.fn(*self.args)
        self.jax.block_until_ready(outs)
        t1 = time.perf_counter()
        return (t1 - t0) / iters * 1e9


@functools.lru_cache(maxsize=None)
def _get_runner(rep=1):
    return _Runner(_get_nc(rep))


def kernel(x, w_qkv, w_proj):
    x = np.asarray(x, dtype=np.float32)
    w_qkv = np.asarray(w_qkv, dtype=np.float32)
    w_proj = np.asarray(w_proj, dtype=np.float32)
    runner = _get_runner()
    runner.put_inputs(make_in_maps(x, w_qkv, w_proj))
    return combine(runner.run())


# revision 3
# speedup vs baseline: 1.0497x; 1.0497x over previous
"""Causal self-attention (B=4, T=2048, C=1024, H=16) on 8 trn2 NeuronCores — v2.

Sharding: core = (batch b, head-group hg), b = core//2, hg = core%2; Megatron
column-parallel qkv / row-parallel proj; host sums the two partial outputs.

v2 layout/schedule choices:
  - all matmul operands bf16 (host pre-converts); fp32 PSUM accumulation.
    Output DMA'd as bf16, host converts+sums.
  - causal handled by ragged diagonal chunks: S^T computed per [128 kv x n q]
    with n = 512-128r on the diagonal — no dead compute, no dead exp, no
    memsets. A 0/1 triangle multiply fixes the single straddling 128-strip.
  - exp is the only ACT work (one instruction per kv chunk, both heads via a
    strided [128,2,n] AP); PSUM->SBUF copies and masks on DVE; softmax
    reciprocal broadcast via Pool-engine partition_broadcast.
  - software pipelining: V-projection chunks and the NEXT pair's Q/K
    projection groups are emitted as PE filler inside each pair's attention
    chunk loop, so PE stays dense while ACT exps (keeps HAM warm).
  - PSUM budget (8 banks): 2 acc (proj) + 2x2 S tiles + 2 y accumulators.
"""

import functools

import numpy as np

B, T, C, H = 4, 2048, 1024, 16
HD = C // H  # 64
N_CORES = 8
HG = 2  # head groups
NH = H // HG  # heads per core = 8
NP = NH // 2  # head pairs per core = 4
TT = T // 128  # 16 kv/t chunks
TB = T // 512  # 4 q blocks
CK = C // 128  # 8 c-chunks


def _build(rep=1):
    import concourse.mybir as mybir
    import concourse.tile as tile
    from concourse import bacc

    f32 = mybir.dt.float32
    bf16 = mybir.dt.bfloat16
    EXP = mybir.ActivationFunctionType.Exp

    nc = bacc.Bacc("TRN2", target_bir_lowering=False, debug=False)

    xt_d = nc.dram_tensor("xt", [C, T], bf16, kind="ExternalInput")
    wq_d = nc.dram_tensor("wq", [C, 512], bf16, kind="ExternalInput")
    wk_d = nc.dram_tensor("wk", [C, 512], bf16, kind="ExternalInput")
    wv_d = nc.dram_tensor("wv", [C, 512], bf16, kind="ExternalInput")
    wp_d = nc.dram_tensor("wp", [512, C], bf16, kind="ExternalInput")
    tri_d = nc.dram_tensor("tri", [128, 128], bf16, kind="ExternalInput")
    out_d = nc.dram_tensor("out", [T, C], bf16, kind="ExternalOutput")

    with tile.TileContext(nc) as tc:
        with tc.tile_pool(name="persist", bufs=1) as persist:
            xt_sb = persist.tile([128, CK, T], bf16, tag="xt")
            wq_sb = persist.tile([128, CK, 512], bf16, tag="wq")
            wk_sb = persist.tile([128, CK, 512], bf16, tag="wk")
            wv_sb = persist.tile([128, CK, 512], bf16, tag="wv")
            wp_sb = persist.tile([128, NP, C], bf16, tag="wp")
            tri_sb = persist.tile([128, 128], bf16, tag="tri")
            qt_sb = persist.tile([128, NP, T], bf16, tag="qt")
            kt_sb = persist.tile([128, NP, T], bf16, tag="kt")
            v_sb = persist.tile([128, TT, NH, 2 * HD], bf16, tag="v")
            yt_sb = persist.tile([128, NP, T], bf16, tag="yt")

            def body():
                # ones column of V' (only column HD; V overlays 0:HD)
                nc.vector.memset(v_sb[:, :, :, HD : HD + 1], 1.0)
                # DMAs in consumption order; xt split by t-quarter so the
                # first V-projection chunks can start early
                for k in range(CK):
                    ksl = slice(k * 128, (k + 1) * 128)
                    nc.sync.dma_start(wv_sb[:, k, :], wv_d[ksl, :])
                    nc.sync.dma_start(xt_sb[:, k, 0:512], xt_d[ksl, 0:512])
                for tq in range(1, 4):
                    tsl = slice(tq * 512, (tq + 1) * 512)
                    for k in range(CK):
                        ksl = slice(k * 128, (k + 1) * 128)
                        nc.sync.dma_start(xt_sb[:, k, tsl], xt_d[ksl, tsl])
                for k in range(CK):
                    ksl = slice(k * 128, (k + 1) * 128)
                    nc.sync.dma_start(wq_sb[:, k, :], wq_d[ksl, :])
                    nc.sync.dma_start(wk_sb[:, k, :], wk_d[ksl, :])
                nc.sync.dma_start(tri_sb[:], tri_d[:, :])
                nc.sync.dma_start(
                    wp_sb[:], wp_d.rearrange("(a p) n -> p a n", p=128)
                )

                with (
                    tc.tile_pool(name="ptp", bufs=6) as ptp,
                    tc.tile_pool(name="recp", bufs=4) as recp,
                    tc.tile_pool(name="outp", bufs=4) as outp,
                    tc.tile_pool(name="psacc", bufs=1, space="PSUM") as psacc,
                    tc.tile_pool(name="pss", bufs=2, space="PSUM") as pss,
                    tc.tile_pool(name="psya", bufs=2, space="PSUM") as psya,
                    tc.tile_pool(name="psyb", bufs=1, space="PSUM") as psyb,
                ):
                    # warm the exp table before any real dependency exists
                    warm = recp.tile([1, 8], f32, tag="warm", name="warm_in")
                    warm2 = recp.tile([1, 8], f32, tag="warm2", name="warm_out")
                    nc.vector.memset(warm[:], 0.0)
                    nc.scalar.activation(warm2[:], warm[:], EXP, scale=1.0)

                    # ---- emitters ----------------------------------------
                    # B and D cycle the "s"-tag [128,1024] slots (pss) so the
                    # single psacc bank is enough for the A-filler groups.
                    def emit_b(tt):  # V projection chunk tt
                        psv = pss.tile([128, 1024], f32, tag="s", name=f"psv{tt}")
                        for k in range(CK):
                            nc.tensor.matmul(
                                psv[:, 0:512],
                                xt_sb[:, k, tt * 128 : (tt + 1) * 128],
                                wv_sb[:, k, :],
                                start=(k == 0), stop=(k == CK - 1),
                            )
                        nc.vector.tensor_copy(
                            v_sb[:, tt, :, 0:HD],
                            psv[:, 0:512].rearrange("p (h e) -> p h e", e=HD),
                        )

                    def emit_a(p, tb, which):  # Q/K projection group
                        w_sb, dst = (wq_sb, qt_sb) if which == 0 else (wk_sb, kt_sb)
                        ps = psacc.tile(
                            [128, 512], f32, tag="acc", name=f"ps{which}_{p}_{tb}"
                        )
                        for k in range(CK):
                            nc.tensor.matmul(
                                ps[:],
                                w_sb[:, k, p * 128 : (p + 1) * 128],
                                xt_sb[:, k, tb * 512 : (tb + 1) * 512],
                                start=(k == 0), stop=(k == CK - 1),
                            )
                        nc.vector.tensor_copy(dst[:, p, tb * 512 : (tb + 1) * 512], ps[:])

                    def chunks_of():
                        return [(qb, c) for qb in range(TB) for c in range(4 * qb + 4)]

                    pt_store = {}
                    y_store = {}

                    def emit_s(p, qb, c):
                        r = c - 4 * qb  # >= 0: diagonal chunk
                        n = 512 if r < 0 else 512 - 128 * r
                        qoff = 512 * qb + (512 - n)
                        ksl = slice(c * 128, (c + 1) * 128)
                        s2 = pss.tile([128, 1024], f32, tag="s", name=f"s2_{p}_{qb}_{c}")
                        # head a -> bank 0 [0:n], head b -> bank 1 [512:512+n]
                        nc.tensor.matmul(
                            s2[:, 0:n],
                            kt_sb[0:64, p, ksl],
                            qt_sb[0:64, p, qoff : qoff + n],
                            start=True, stop=True,
                        )
                        nc.tensor.matmul(
                            s2[:, 512 : 512 + n],
                            kt_sb[64:128, p, ksl],
                            qt_sb[64:128, p, qoff : qoff + n],
                            start=True, stop=True,
                        )
                        pt = ptp.tile([128, 1024], bf16, tag="pt", name=f"pt_{p}_{qb}_{c}")
                        s2v = s2[:].rearrange("p (b x) -> p b x", b=2)[:, :, 0:n]
                        ptv = pt[:].rearrange("p (b x) -> p b x", b=2)[:, :, 0:n]
                        nc.scalar.activation(ptv, s2v, EXP, scale=0.125)
                        if r >= 0:
                            nc.vector.tensor_mul(pt[:, 0:128], pt[:, 0:128], tri_sb[:])
                            nc.vector.tensor_mul(
                                pt[:, 512:640], pt[:, 512:640], tri_sb[:]
                            )
                        pt_store[(p, qb, c)] = (pt, n)

                    def emit_av(p, qb, c):
                        pt, n = pt_store.pop((p, qb, c))
                        qloc = 512 - n
                        if (p, qb) not in y_store:
                            ya = psya.tile([128, 512], f32, tag="ya", name=f"ya_{p}_{qb}")
                            yb = psyb.tile([128, 512], f32, tag="yb", name=f"yb_{p}_{qb}")
                            y_store[(p, qb)] = (ya, yb)
                        ya, yb = y_store[(p, qb)]
                        st, sp = c == 0, c == 4 * qb + 3
                        nc.tensor.matmul(
                            ya[:, qloc:512], v_sb[:, c, 2 * p, :], pt[:, 0:n],
                            start=st, stop=sp,
                        )
                        nc.tensor.matmul(
                            yb[:, qloc:512], v_sb[:, c, 2 * p + 1, :],
                            pt[:, 512 : 512 + n],
                            start=st, stop=sp,
                        )
                        if not sp:
                            return
                        qsl = slice(qb * 512, (qb + 1) * 512)
                        for hi, yy in ((0, ya), (1, yb)):
                            ri = recp.tile(
                                [1, 512], f32, tag="ri", name=f"ri_{p}_{qb}_{hi}"
                            )
                            nc.vector.tensor_copy(ri[:], yy[64:65, :])
                            rec = recp.tile(
                                [1, 512], f32, tag="rec", name=f"rec_{p}_{qb}_{hi}"
                            )
                            nc.vector.reciprocal_approx_fast(rec[:], ri[:])
                            bc = recp.tile(
                                [64, 512], f32, tag="bc", name=f"bc_{p}_{qb}_{hi}"
                            )
                            nc.gpsimd.partition_broadcast(bc[:], rec[:])
                            nc.vector.tensor_mul(
                                yt_sb[hi * 64 : (hi + 1) * 64, p, qsl],
                                yy[0:64, :], bc[:],
                            )
                        del y_store[(p, qb)]

                    def emit_d(tt, nb, acc_only=False):
                        if acc_only or (2 * tt + nb) % 3 == 2:
                            pot = psacc.tile(
                                [128, 512], f32, tag="acc", name=f"po{tt}_{nb}"
                            )
                            po = pot[:]
                        else:
                            pot = pss.tile(
                                [128, 1024], f32, tag="s", name=f"po{tt}_{nb}"
                            )
                            po = pot[:, 0:512]
                        for p in range(NP):
                            nc.tensor.matmul(
                                po,
                                yt_sb[:, p, tt * 128 : (tt + 1) * 128],
                                wp_sb[:, p, nb * 512 : (nb + 1) * 512],
                                start=(p == 0), stop=(p == NP - 1),
                            )
                        ot = outp.tile([128, 512], bf16, tag="ot", name=f"ot{tt}_{nb}")
                        if (2 * tt + nb) % 2:
                            nc.scalar.copy(ot[:], po)
                        else:
                            nc.vector.tensor_copy(ot[:], po)
                        nc.sync.dma_start(
                            out_d[tt * 128 : (tt + 1) * 128, nb * 512 : (nb + 1) * 512],
                            ot[:],
                        )

                    # ---- schedule emission -------------------------------
                    LA = 2

                    def run_pair(p, fillers, d_after_qb=False):
                        cl = chunks_of()
                        drop = {}
                        if fillers:
                            for i, f in enumerate(fillers):
                                drop.setdefault(int(i * len(cl) / len(fillers)), []).append(f)
                        for j in range(min(LA, len(cl))):
                            emit_s(p, *cl[j])
                        for i in range(len(cl)):
                            for f in drop.get(i, ()):
                                f()
                            if i + LA < len(cl):
                                emit_s(p, *cl[i + LA])
                            emit_av(p, *cl[i])
                            qb, c = cl[i]
                            if d_after_qb and c == 4 * qb + 3 and qb < TB - 1:
                                for tt in range(4 * qb, 4 * qb + 4):
                                    for nb in range(2):
                                        emit_d(tt, nb, acc_only=True)

                    # prolog: alternate pair-0 Q/K groups with the first 8 V
                    # chunks; the remaining V chunks become pair-0 filler
                    for j in range(8):
                        emit_a(0, j // 2, j % 2)
                        emit_b(j)
                    run_pair(
                        0,
                        [functools.partial(emit_b, tt) for tt in range(8, TT)]
                        + [
                            functools.partial(emit_a, 1, tb, w)
                            for tb in range(TB) for w in (0, 1)
                        ],
                    )
                    run_pair(
                        1,
                        [
                            functools.partial(emit_a, 2, tb, w)
                            for tb in range(TB) for w in (0, 1)
                        ],
                    )
                    run_pair(
                        2,
                        [
                            functools.partial(emit_a, 3, tb, w)
                            for tb in range(TB) for w in (0, 1)
                        ],
                    )
                    run_pair(3, [], d_after_qb=True)
                    for tt in range(12, TT):
                        for nb in range(2):
                            emit_d(tt, nb)

            if rep == 1:
                body()
            else:
                with tc.For_i(0, rep, 1):
                    body()

    nc.compile()
    return nc


@functools.lru_cache(maxsize=None)
def _get_nc(rep=1):
    return _build(rep)


def make_in_maps(x, w_qkv, w_proj):
    import ml_dtypes

    bf = ml_dtypes.bfloat16
    tri = (np.arange(128)[None, :] >= np.arange(128)[:, None]).astype(bf)
    in_maps = []
    for core in range(N_CORES):
        b, hg = divmod(core, HG)
        sl = slice(hg * 512, (hg + 1) * 512)
        in_maps.append({
            "xt": np.ascontiguousarray(x[b].T).astype(bf),
            "wq": np.ascontiguousarray(w_qkv[sl].T).astype(bf),
            "wk": np.ascontiguousarray(w_qkv[C : 2 * C][sl].T).astype(bf),
            "wv": np.ascontiguousarray(w_qkv[2 * C : 3 * C][sl].T).astype(bf),
            "wp": np.ascontiguousarray(w_proj[:, sl].T).astype(bf),
            "tri": tri,
        })
    return in_maps


def combine(results):
    out = np.empty((B, T, C), dtype=np.float32)
    for b in range(B):
        out[b] = results[2 * b]["out"].astype(np.float32) + results[
            2 * b + 1
        ]["out"].astype(np.float32)
    return out


# ---------------------------------------------------------------------------
# PJRT runner (device-resident inputs, reusable jitted executable)
# ---------------------------------------------------------------------------

class _Runner:
    def __init__(self, nc, n_cores=N_CORES):
        import jax
        import concourse.mybir as mybir
        from concourse import bass2jax
        from jax.sharding import Mesh, PartitionSpec, NamedSharding
        from jax.experimental.shard_map import shard_map

        self.jax = jax
        bass2jax.install_neuronx_cc_hook()
        partition_name = (
            nc.partition_id_tensor.name if nc.partition_id_tensor else None
        )
        in_names, out_names, out_avals, zero_outs = [], [], [], []
        for alloc in nc.m.functions[0].allocations:
            if not isinstance(alloc, mybir.MemoryLocationSet):
                continue
            name = alloc.memorylocations[0].name
            if alloc.kind == "ExternalInput":
                if name != partition_name:
                    in_names.append(name)
            elif alloc.kind == "ExternalOutput":
                out_names.append(name)
                shape = tuple(alloc.tensor_shape)
                dtype = mybir.dt.np(alloc.dtype)
                out_avals.append(jax.core.ShapedArray(shape, dtype))
                zero_outs.append(np.zeros(shape, dtype))
        self.in_names, self.out_names = in_names, out_names
        self.out_avals, self.zero_outs = out_avals, zero_outs
        self.n_cores = n_cores
        all_names = in_names + out_names
        if partition_name is not None:
            all_names = all_names + [partition_name]

        def _bdy(*args):
            operands = list(args)
            if partition_name is not None:
                operands.append(bass2jax.partition_id_tensor())
            outs = bass2jax._bass_exec_p.bind(
                *operands,
                out_avals=tuple(out_avals),
                in_names=tuple(all_names),
                out_names=tuple(out_names),
                lowering_input_output_aliases=(),
                sim_require_finite=True,
                sim_require_nnan=True,
                nc=nc,
            )
            return tuple(outs)

        devices = jax.devices()[:n_cores]
        mesh = Mesh(np.asarray(devices), ("core",))
        n_args = len(in_names) + len(out_names)
        self.fn = jax.jit(
            shard_map(
                _bdy, mesh=mesh,
                in_specs=(PartitionSpec("core"),) * n_args,
                out_specs=(PartitionSpec("core"),) * len(out_names),
                check_rep=False,
            ),
            keep_unused=True,
        )
        self.sharding = NamedSharding(mesh, PartitionSpec("core"))

    def put_inputs(self, in_maps):
        concat = [
            np.concatenate([np.asarray(m[name]) for m in in_maps], axis=0)
            for name in self.in_names
        ]
        concat += [
            np.zeros((self.n_cores * z.shape[0], *z.shape[1:]), z.dtype)
            for z in self.zero_outs
        ]
        self.args = [self.jax.device_put(a, self.sharding) for a in concat]
        self.jax.block_until_ready(self.args)

    def run(self):
        outs = self.fn(*self.args)
        self.jax.block_until_ready(outs)
        return [
            {
                name: np.asarray(outs[i]).reshape(
                    self.n_cores, *self.out_avals[i].shape)[c]
                for i, name in enumerate(self.out_names)
            }
            for c in range(self.n_cores)
        ]

    def time_ns(self, iters=20, warmup=2):
        import time
        for _ in range(warmup):
            self.jax.block_until_ready(self.fn(*self.args))
        t0 = time.perf_counter()
        outs = None
        for _ in range(iters):
            outs = self.fn(*self.args)
        self.jax.block_until_ready(outs)
        t1 = time.perf_counter()
        return (t1 - t0) / iters * 1e9


@functools.lru_cache(maxsize=None)
def _get_runner(rep=1):
    return _Runner(_get_nc(rep))


def kernel(x, w_qkv, w_proj):
    x = np.asarray(x, dtype=np.float32)
    w_qkv = np.asarray(w_qkv, dtype=np.float32)
    w_proj = np.asarray(w_proj, dtype=np.float32)
    runner = _get_runner()
    runner.put_inputs(make_in_maps(x, w_qkv, w_proj))
    return combine(runner.run())
